# revision 20
# baseline (speedup 1.0000x reference)
"""3D Haar DWT (clean-mode subband stack) on 8 Trainium2 NeuronCores.

Problem (hardcoded): inputs (4, 128, 128, 128, 4) f32, A (128, 128) f32 Haar
analysis operator. Output (4, 64, 64, 64, 32) f32 = 8 subbands stacked on the
channel axis (LLL, LLH, LHL, LHH, HLL, HLH, HHL, HHH) x 4 channels.

Sharding: pure data parallel over (batch, d1-half): core k handles
b = k // 2, d1 range [64*(k%2), 64*(k%2)+64). The Haar transform is a 2-tap
non-overlapping filter, so splitting d1 on an even boundary requires no
communication.

The kernel is memory-bound (HBM ~358 GB/s per core), so the datapath runs in
bf16: the host uploads the input slab as bf16 (8 MiB/core instead of 16) and
the output is stored as bf16 (8 MiB instead of 16), halving HBM traffic vs
f32. absmax-relative error lands ~7e-3, inside the 2e-2 gate.

Key structure: the partition axis carries (d1_local, d3_parity), so a SINGLE
PE pass applies BOTH the d1 and d3 butterflies at once — the stationary
matrix is the Kronecker product of the two 2-tap Haar stages (entries
+-0.25, bf16-exact, loaded once). Only the d2 butterfly remains as an
elementwise pass (DVE, free axis), plus the mandatory one-input PSUM
evacuation (the ISA allows at most one PSUM operand per elementwise op),
which is split 3/4 ACT : 1/4 DVE to balance engine time. GpSimd does no
compute (its tensor ops measure 3.4x slower than DVE and its SBUF traffic
degrades DVE throughput).

Per-core pipeline (host layout [(d1l, d3par) = 128, o3 64, d2 128, c 4]):
  1. DMA in 1 MiB chunks (8 o3 values) on the SP HWDGE ring, 8 KiB
     descriptors; all loads enqueued before any store so stores can never
     head-of-line-block a load.
  2. PE: 8 x 512-col matmuls per chunk (one per o3 value) with the Kronecker
     stationary; PSUM partition axis becomes (s1, s3, o1_local).
  3. PSUM evacuation (one-input copy, f32 -> bf16): one op per half-chunk,
     3 on ACT : 1 on DVE.
  4. d2 butterfly on DVE (bf16 TT, 2 elem/cycle) -> (o3, s2, o2, c) layout.
  5. One 1 MiB store per chunk (8 KiB runs) on the SP ring; the output
     partition axis is already subband-major, so the host just casts and
     transposes.

Scale bookkeeping: reference applies s = 1/sqrt(2) per axis (s^3 total). The
host pre-scales by sqrt(2), the PE applies 0.25, the d2 butterfly +-1:
sqrt(2) * 0.25 = s^3 — exact.
"""

import sys

import numpy as np

if "/opt/trn_rl_repo" not in sys.path:
    sys.path.insert(0, "/opt/trn_rl_repo")

B, N, C = 4, 128, 4
N_CORES = 8
SLAB = 64          # d1 extent per core
LO3 = 16           # o3 values per load (2 MiB transfers, 16 KiB runs)
O3C = 8            # o3 values per compute block / store (1 MiB stores)
NLOAD = 64 // LO3
NBLK = 64 // O3C

_BASS_CACHE = {}


def _haar_matrix():
    s = np.float32(1.0 / np.sqrt(2.0))
    A = np.zeros((N, N), dtype=np.float32)
    for i in range(N // 2):
        A[i, 2 * i] = s
        A[i, 2 * i + 1] = s
        A[64 + i, 2 * i] = -s
        A[64 + i, 2 * i + 1] = s
    return A


def _kron_weights():
    """lhsT [p_in, p_out] for the combined (d1, d3-parity) butterfly.

    p_in  = 2 * d1l + m3, d1l = 2 * o1l + m1  (input partition order)
    p_out = 64 * s1 + 32 * s3 + o1l           (output partition order)
    weight = 0.25 * g(s1, m1) * g(s3, m3), g(0,m)=+1, g(1,0)=-1, g(1,1)=+1.
    """
    g = np.array([[1.0, 1.0], [-1.0, 1.0]], dtype=np.float32)
    lhsT = np.zeros((N, N), dtype=np.float32)
    for o1l in range(32):
        for m1 in range(2):
            for m3 in range(2):
                p_in = 2 * (2 * o1l + m1) + m3
                for s1 in range(2):
                    for s3 in range(2):
                        p_out = 64 * s1 + 32 * s3 + o1l
                        lhsT[p_in, p_out] = 0.25 * g[s1, m1] * g[s3, m3]
    return lhsT


def _reference_numpy(inputs, A):
    # Fallback only: exact reference math on host (used if A is not Haar).
    x = np.einsum("ij,bpjqc->bpiqc", A, inputs)
    x = np.einsum("ij,bjpqc->bipqc", A, x)
    x = np.einsum("ij,bpqjc->bpqic", A, x)
    m = x.shape[1] // 2
    subs = [
        x[:, :m, :m, :m, :], x[:, :m, :m, m:, :],
        x[:, :m, m:, :m, :], x[:, :m, m:, m:, :],
        x[:, m:, :m, :m, :], x[:, m:, :m, m:, :],
        x[:, m:, m:, :m, :], x[:, m:, m:, m:, :],
    ]
    return np.concatenate(subs, axis=-1).astype(np.float32)


def _build_bass():
    import concourse.bacc as bacc
    import concourse.mybir as mybir
    import concourse.tile as tile

    f32 = mybir.dt.float32
    bf16 = mybir.dt.bfloat16

    # Bacc (not raw Bass): its compile() pipeline splits multi-sem waits into
    # EventSemaphore instructions — TRN2 instructions have one wait slot.
    nc = bacc.Bacc("TRN2", target_bir_lowering=False, debug=False)
    # x host layout: [(d1l, d3par), o3, d2, c] so each load descriptor covers
    # an 8 KiB contiguous run per partition.
    x = nc.dram_tensor("x", [N, 64, N, C], bf16, kind="ExternalInput")
    wk = nc.dram_tensor("wk", [N, N], bf16, kind="ExternalInput")
    # y: [(s1, s3, o1l) = 128, o3, s2, o2, c]; per-partition contiguous run
    # for one chunk's o3 range = 8 KiB.
    y = nc.dram_tensor("y", [N, 64, 2, 64, C], bf16, kind="ExternalOutput")

    with tile.TileContext(nc) as tc:
        with (
            tc.tile_pool(name="const", bufs=1) as cpool,
            tc.tile_pool(name="io", bufs=4) as tpool,
            tc.tile_pool(name="emid", bufs=3) as mpool,
            tc.tile_pool(name="wmid", bufs=4) as wpool,
            tc.tile_pool(name="psum", bufs=2, space="PSUM") as ppool,
        ):
            wk_sb = cpool.tile([N, N], bf16)

            # 1. all loads enqueued up-front on the SP ring: none depends on
            # compute, so the load stream runs back-to-back from the end of
            # the preamble. 2 MiB transfers (16 KiB descriptor runs) sit
            # higher on the DMA size-efficiency curve than 1 MiB.
            Ts = []
            for li in range(NLOAD):
                T = tpool.tile([N, LO3, N * C], bf16, tag="T")
                nc.sync.dma_start(
                    out=T[:],
                    in_=x[:, li * LO3:(li + 1) * LO3].rearrange(
                        "p a q c -> p a (q c)"
                    ),
                )
                Ts.append(T)
                if li == 0:
                    nc.sync.dma_start(out=wk_sb[:], in_=wk[:, :])

            evac_t = 0
            for ci in range(NBLK):
                T = Ts[ci * O3C // LO3]
                off = (ci * O3C) % LO3
                # E: evacuated matmul output, (p_out, o3, d2*c) bf16.
                E = mpool.tile([N, O3C, N * C], bf16, tag="E")
                for hc in range(2):
                    # 2. combined (d1 x d3-parity) butterfly as matmul: one
                    # 512-col matmul per o3 value -> one PSUM bank each (the
                    # ISA caps the moving operand at 512 cols for f32 PSUM).
                    ps = ppool.tile([N, 4, 512], f32, tag="ps")
                    for j in range(4):
                        nc.tensor.matmul(
                            ps[:, j],
                            lhsT=wk_sb[:],
                            rhs=T[:, off + 4 * hc + j],
                            start=True, stop=True,
                        )
                    # 3. one-input PSUM evacuation (f32 -> bf16), one op per
                    # half-chunk. Blocks 5-6 on DVE, the rest on ACT: the
                    # totals balance (ACT ~28us, DVE ~29us), and the LAST
                    # block's evacs stay on ACT so they run concurrently
                    # with DVE's trailing butterflies.
                    dst = E[:, 4 * hc:4 * hc + 4]
                    if NBLK - 3 <= ci <= NBLK - 2:
                        nc.vector.tensor_copy(out=dst, in_=ps[:])
                    else:
                        nc.scalar.copy(out=dst, in_=ps[:])
                    evac_t += 1

                # 4. d2 butterfly on DVE: W[..., s2=0, o2, c] = even + odd,
                # s2=1: odd - even; layout (p_out, o3, s2, o2, c).
                # 5. one 1 MiB store per block (8 KiB runs/partition) on the
                # SP ring, enqueued behind all loads. The last block is split
                # in two (0.5 MiB stores, 4 KiB runs) so the final store
                # starts as early as possible — it is the kernel's tail.
                W = wpool.tile([N, O3C, 2, 64, C], bf16, tag="W")
                Ev = E[:].rearrange("p a (o t c) -> p a o t c", t=2, c=C)
                nsub = 2 if ci == NBLK - 1 else 1
                for sub in range(nsub):
                    o3a = sub * O3C // nsub
                    o3b = (sub + 1) * O3C // nsub
                    nc.vector.tensor_add(
                        out=W[:, o3a:o3b, 0],
                        in0=Ev[:, o3a:o3b, :, 0], in1=Ev[:, o3a:o3b, :, 1],
                    )
                    nc.vector.tensor_sub(
                        out=W[:, o3a:o3b, 1],
                        in0=Ev[:, o3a:o3b, :, 1], in1=Ev[:, o3a:o3b, :, 0],
                    )
                    nc.sync.dma_start(
                        out=y[:, ci * O3C + o3a:ci * O3C + o3b].rearrange(
                            "p a t q c -> p a (t q c)"
                        ),
                        in_=W[:, o3a:o3b].rearrange("p a t q c -> p a (t q c)"),
                    )
    nc.compile()
    return nc


def _prepare(x, A):
    """Host-side prep shared with test.py: build (nc, in_maps)."""
    import ml_dtypes

    if "nc" not in _BASS_CACHE:
        _BASS_CACHE["nc"] = _build_bass()
    nc = _BASS_CACHE["nc"]

    wk = np.ascontiguousarray(_kron_weights().astype(ml_dtypes.bfloat16))
    # pre-scale by sqrt(2): PE applies 0.25 and the d2 butterfly +-1, so each
    # path nets sqrt(2)/4 = (1/sqrt(2))^3.
    xb = (x * np.float32(np.sqrt(2.0))).astype(ml_dtypes.bfloat16)
    in_maps = []
    for k in range(N_CORES):
        b, h = divmod(k, 2)
        # slab [d1l 64, d2 128, d3 128, c] -> [(d1l, m3) 128, o3 64, d2, c]
        s = xb[b, h * SLAB:(h + 1) * SLAB]            # (64, 128, 128, 4)
        s = s.reshape(SLAB, N, 64, 2, C)              # (d1l, d2, o3, m3, c)
        s = s.transpose(0, 3, 2, 1, 4)                # (d1l, m3, o3, d2, c)
        in_maps.append(
            {
                "x": np.ascontiguousarray(s.reshape(N, 64, N, C)),
                "wk": wk,
            }
        )
    return nc, in_maps


def _assemble(results):
    """Gather per-core bf16 y tensors into the full f32 output."""
    out = np.empty((B, 64, 64, 64, 8 * C), np.float32)
    for k in range(N_CORES):
        b, h = divmod(k, 2)
        # y: [(s1, s3, o1l), o3, s2, o2, c]
        arr = results[k]["y"].astype(np.float32).reshape(2, 2, 32, 64, 2, 64, C)
        # (s1, s3, o1l, o3, s2, o2, c) -> (o1l, o2, o3, s1, s2, s3, c)
        out[b, 32 * h:32 * h + 32] = (
            arr.transpose(2, 5, 3, 0, 4, 1, 6).reshape(32, 64, 64, 8 * C)
        )
    return out


def kernel(**inputs):
    x = np.ascontiguousarray(np.asarray(inputs["inputs"], dtype=np.float32))
    A = np.asarray(inputs["A"], dtype=np.float32)
    assert x.shape == (B, N, N, N, C), x.shape

    if not np.allclose(A, _haar_matrix(), atol=1e-5):
        # Kernel hardcodes the 2-tap Haar structure; fall back for generic A.
        return _reference_numpy(x, A)

    from concourse.bass_utils import run_bass_kernel_spmd

    nc, in_maps = _prepare(x, A)
    res = run_bass_kernel_spmd(nc, in_maps, core_ids=list(range(N_CORES)))
    return _assemble(res.results)


# revision 21
# speedup vs baseline: 1.0905x; 1.0905x over previous
"""3D Haar DWT (clean-mode subband stack) on 8 Trainium2 NeuronCores.

Problem (hardcoded): inputs (4, 128, 128, 128, 4) f32, A (128, 128) f32 Haar
analysis operator. Output (4, 64, 64, 64, 32) f32 = 8 subbands stacked on the
channel axis (LLL, LLH, LHL, LHH, HLL, HLH, HHL, HHH) x 4 channels.

Sharding: pure data parallel over (batch, d1-half): core k handles
b = k // 2, d1 range [64*(k%2), 64*(k%2)+64). The Haar transform is a 2-tap
non-overlapping filter, so splitting d1 on an even boundary requires no
communication.

The kernel is memory-bound (HBM ~358 GB/s per core), so the datapath runs in
bf16: the host uploads the input slab as bf16 (8 MiB/core instead of 16) and
the output is stored as bf16 (8 MiB instead of 16), halving HBM traffic vs
f32. absmax-relative error lands ~7e-3, inside the 2e-2 gate.

Key structure: the partition axis carries (d1_local, d3_parity), so a SINGLE
PE pass applies BOTH the d1 and d3 butterflies at once — the stationary
matrix is the Kronecker product of the two 2-tap Haar stages (entries
+-0.25, bf16-exact, loaded once). Only the d2 butterfly remains as an
elementwise pass (DVE, free axis), plus the mandatory one-input PSUM
evacuation (the ISA allows at most one PSUM operand per elementwise op),
which is split 3/4 ACT : 1/4 DVE to balance engine time. GpSimd does no
compute (its tensor ops measure 3.4x slower than DVE and its SBUF traffic
degrades DVE throughput).

Per-core pipeline (host layout [(d1l, d3par) = 128, o3 64, d2 128, c 4]):
  1. DMA in 1 MiB chunks (8 o3 values) on the SP HWDGE ring, 8 KiB
     descriptors; all loads enqueued before any store so stores can never
     head-of-line-block a load.
  2. PE: 8 x 512-col matmuls per chunk (one per o3 value) with the Kronecker
     stationary; PSUM partition axis becomes (s1, s3, o1_local).
  3. PSUM evacuation (one-input copy, f32 -> bf16): one op per half-chunk,
     3 on ACT : 1 on DVE.
  4. d2 butterfly on DVE (bf16 TT, 2 elem/cycle) -> (o3, s2, o2, c) layout.
  5. One 1 MiB store per chunk (8 KiB runs) on the SP ring; the output
     partition axis is already subband-major, so the host just casts and
     transposes.

Scale bookkeeping: reference applies s = 1/sqrt(2) per axis (s^3 total). The
host pre-scales by sqrt(2), the PE applies 0.25, the d2 butterfly +-1:
sqrt(2) * 0.25 = s^3 — exact.
"""

import sys

import numpy as np

if "/opt/trn_rl_repo" not in sys.path:
    sys.path.insert(0, "/opt/trn_rl_repo")

B, N, C = 4, 128, 4
N_CORES = 8
SLAB = 64          # d1 extent per core
LO3 = 16           # o3 values per load (2 MiB transfers, 16 KiB runs)
O3C = 8            # o3 values per compute block / store (1 MiB stores)
NLOAD = 64 // LO3
NBLK = 64 // O3C

_BASS_CACHE = {}


def _haar_matrix():
    s = np.float32(1.0 / np.sqrt(2.0))
    A = np.zeros((N, N), dtype=np.float32)
    for i in range(N // 2):
        A[i, 2 * i] = s
        A[i, 2 * i + 1] = s
        A[64 + i, 2 * i] = -s
        A[64 + i, 2 * i + 1] = s
    return A


def _kron_weights():
    """lhsT [p_in, p_out] for the combined (d1, d3-parity) butterfly.

    p_in  = 2 * d1l + m3, d1l = 2 * o1l + m1  (input partition order)
    p_out = 64 * s1 + 32 * s3 + o1l           (output partition order)
    weight = 0.25 * g(s1, m1) * g(s3, m3), g(0,m)=+1, g(1,0)=-1, g(1,1)=+1.
    """
    g = np.array([[1.0, 1.0], [-1.0, 1.0]], dtype=np.float32)
    lhsT = np.zeros((N, N), dtype=np.float32)
    for o1l in range(32):
        for m1 in range(2):
            for m3 in range(2):
                p_in = 2 * (2 * o1l + m1) + m3
                for s1 in range(2):
                    for s3 in range(2):
                        p_out = 64 * s1 + 32 * s3 + o1l
                        lhsT[p_in, p_out] = 0.25 * g[s1, m1] * g[s3, m3]
    return lhsT


def _reference_numpy(inputs, A):
    # Fallback only: exact reference math on host (used if A is not Haar).
    x = np.einsum("ij,bpjqc->bpiqc", A, inputs)
    x = np.einsum("ij,bjpqc->bipqc", A, x)
    x = np.einsum("ij,bpqjc->bpqic", A, x)
    m = x.shape[1] // 2
    subs = [
        x[:, :m, :m, :m, :], x[:, :m, :m, m:, :],
        x[:, :m, m:, :m, :], x[:, :m, m:, m:, :],
        x[:, m:, :m, :m, :], x[:, m:, :m, m:, :],
        x[:, m:, m:, :m, :], x[:, m:, m:, m:, :],
    ]
    return np.concatenate(subs, axis=-1).astype(np.float32)


def _build_bass():
    import concourse.bacc as bacc
    import concourse.mybir as mybir
    import concourse.tile as tile

    f32 = mybir.dt.float32
    bf16 = mybir.dt.bfloat16

    # Bacc (not raw Bass): its compile() pipeline splits multi-sem waits into
    # EventSemaphore instructions — TRN2 instructions have one wait slot.
    nc = bacc.Bacc("TRN2", target_bir_lowering=False, debug=False)
    # x host layout: [(d1l, d3par), o3, d2, c] so each load descriptor covers
    # an 8 KiB contiguous run per partition.
    x = nc.dram_tensor("x", [N, 64, N, C], bf16, kind="ExternalInput")
    wk = nc.dram_tensor("wk", [N, N], bf16, kind="ExternalInput")
    # y: [(s1, s3, o1l) = 128, o3, s2, o2, c]; per-partition contiguous run
    # for one chunk's o3 range = 8 KiB.
    y = nc.dram_tensor("y", [N, 64, 2, 64, C], bf16, kind="ExternalOutput")

    with tile.TileContext(nc) as tc:
        with (
            tc.tile_pool(name="const", bufs=1) as cpool,
            tc.tile_pool(name="io", bufs=4) as tpool,
            tc.tile_pool(name="emid", bufs=3) as mpool,
            tc.tile_pool(name="wmid", bufs=4) as wpool,
            tc.tile_pool(name="psum", bufs=2, space="PSUM") as ppool,
        ):
            wk_sb = cpool.tile([N, N], bf16)

            # 1. all loads enqueued up-front on the SP ring: none depends on
            # compute, so the load stream runs back-to-back from the end of
            # the preamble. 2 MiB transfers (16 KiB descriptor runs) sit
            # higher on the DMA size-efficiency curve than 1 MiB.
            Ts = []
            for li in range(NLOAD):
                T = tpool.tile([N, LO3, N * C], bf16, tag="T")
                nc.sync.dma_start(
                    out=T[:],
                    in_=x[:, li * LO3:(li + 1) * LO3].rearrange(
                        "p a q c -> p a (q c)"
                    ),
                )
                Ts.append(T)
                if li == 0:
                    nc.sync.dma_start(out=wk_sb[:], in_=wk[:, :])

            evac_t = 0
            for ci in range(NBLK):
                T = Ts[ci * O3C // LO3]
                off = (ci * O3C) % LO3
                # E: evacuated matmul output, (p_out, o3, d2*c) bf16.
                E = mpool.tile([N, O3C, N * C], bf16, tag="E")
                for hc in range(2):
                    # 2. combined (d1 x d3-parity) butterfly as matmul: one
                    # 512-col matmul per o3 value -> one PSUM bank each (the
                    # ISA caps the moving operand at 512 cols for f32 PSUM).
                    ps = ppool.tile([N, 4, 512], f32, tag="ps")
                    for j in range(4):
                        nc.tensor.matmul(
                            ps[:, j],
                            lhsT=wk_sb[:],
                            rhs=T[:, off + 4 * hc + j],
                            start=True, stop=True,
                        )
                    # 3. one-input PSUM evacuation (f32 -> bf16), one op per
                    # half-chunk. The first two blocks' evacs go to DVE
                    # (which would otherwise idle during pipeline fill), the
                    # rest to ACT — so at the tail DVE runs only the short
                    # butterflies and ACT's continuous evac queue ends as
                    # early as possible.
                    dst = E[:, 4 * hc:4 * hc + 4]
                    if ci <= 1:
                        nc.vector.tensor_copy(out=dst, in_=ps[:])
                    else:
                        nc.scalar.copy(out=dst, in_=ps[:])
                    evac_t += 1

                # 4. d2 butterfly on DVE: W[..., s2=0, o2, c] = even + odd,
                # s2=1: odd - even; layout (p_out, o3, s2, o2, c).
                # 5. one 1 MiB store per block (8 KiB runs/partition) on the
                # SP ring, enqueued behind all loads. The last block is split
                # in two (0.5 MiB stores, 4 KiB runs) so the final store
                # starts as early as possible — it is the kernel's tail.
                W = wpool.tile([N, O3C, 2, 64, C], bf16, tag="W")
                Ev = E[:].rearrange("p a (o t c) -> p a o t c", t=2, c=C)
                nsub = 2 if ci == NBLK - 1 else 1
                for sub in range(nsub):
                    o3a = sub * O3C // nsub
                    o3b = (sub + 1) * O3C // nsub
                    nc.vector.tensor_add(
                        out=W[:, o3a:o3b, 0],
                        in0=Ev[:, o3a:o3b, :, 0], in1=Ev[:, o3a:o3b, :, 1],
                    )
                    nc.vector.tensor_sub(
                        out=W[:, o3a:o3b, 1],
                        in0=Ev[:, o3a:o3b, :, 1], in1=Ev[:, o3a:o3b, :, 0],
                    )
                    nc.sync.dma_start(
                        out=y[:, ci * O3C + o3a:ci * O3C + o3b].rearrange(
                            "p a t q c -> p a (t q c)"
                        ),
                        in_=W[:, o3a:o3b].rearrange("p a t q c -> p a (t q c)"),
                    )
    nc.compile()
    return nc


def _prepare(x, A):
    """Host-side prep shared with test.py: build (nc, in_maps)."""
    import ml_dtypes

    if "nc" not in _BASS_CACHE:
        _BASS_CACHE["nc"] = _build_bass()
    nc = _BASS_CACHE["nc"]

    wk = np.ascontiguousarray(_kron_weights().astype(ml_dtypes.bfloat16))
    # pre-scale by sqrt(2): PE applies 0.25 and the d2 butterfly +-1, so each
    # path nets sqrt(2)/4 = (1/sqrt(2))^3.
    xb = (x * np.float32(np.sqrt(2.0))).astype(ml_dtypes.bfloat16)
    in_maps = []
    for k in range(N_CORES):
        b, h = divmod(k, 2)
        # slab [d1l 64, d2 128, d3 128, c] -> [(d1l, m3) 128, o3 64, d2, c]
        s = xb[b, h * SLAB:(h + 1) * SLAB]            # (64, 128, 128, 4)
        s = s.reshape(SLAB, N, 64, 2, C)              # (d1l, d2, o3, m3, c)
        s = s.transpose(0, 3, 2, 1, 4)                # (d1l, m3, o3, d2, c)
        in_maps.append(
            {
                "x": np.ascontiguousarray(s.reshape(N, 64, N, C)),
                "wk": wk,
            }
        )
    return nc, in_maps


def _assemble(results):
    """Gather per-core bf16 y tensors into the full f32 output."""
    out = np.empty((B, 64, 64, 64, 8 * C), np.float32)
    for k in range(N_CORES):
        b, h = divmod(k, 2)
        # y: [(s1, s3, o1l), o3, s2, o2, c]
        arr = results[k]["y"].astype(np.float32).reshape(2, 2, 32, 64, 2, 64, C)
        # (s1, s3, o1l, o3, s2, o2, c) -> (o1l, o2, o3, s1, s2, s3, c)
        out[b, 32 * h:32 * h + 32] = (
            arr.transpose(2, 5, 3, 0, 4, 1, 6).reshape(32, 64, 64, 8 * C)
        )
    return out


def kernel(**inputs):
    x = np.ascontiguousarray(np.asarray(inputs["inputs"], dtype=np.float32))
    A = np.asarray(inputs["A"], dtype=np.float32)
    assert x.shape == (B, N, N, N, C), x.shape

    if not np.allclose(A, _haar_matrix(), atol=1e-5):
        # Kernel hardcodes the 2-tap Haar structure; fall back for generic A.
        return _reference_numpy(x, A)

    from concourse.bass_utils import run_bass_kernel_spmd

    nc, in_maps = _prepare(x, A)
    res = run_bass_kernel_spmd(nc, in_maps, core_ids=list(range(N_CORES)))
    return _assemble(res.results)
